# revision 1
# baseline (speedup 1.0000x reference)
"""BiLSTM classifier kernel for 8 trn2 NeuronCores.

Sharding: pure data parallel — batch B=256 is split 8 ways (32 per core),
LSTM weights are replicated, final logits are gathered on the host.
Self-contained: hardcodes the model structure (3 BiLSTM layers + MLP head).
"""
import numpy as np
import jax
import jax.numpy as jnp
from jax import lax

_PARAM_NAMES = (
    "Wih1", "Whh1", "bih1", "bhh1",
    "Wih2", "Whh2", "bih2", "bhh2",
    "Wih3", "Whh3", "bih3", "bhh3",
    "hW1", "hb1", "hW2", "hb2", "hW3", "hb3",
)


def _lstm_dir(x, Wih, Whh, bih, bhh, reverse):
    """Single-direction LSTM. x: (B,T,D). Wih: (4H,D), Whh: (4H,H)."""
    B = x.shape[0]
    H = Whh.shape[1]
    xs = jnp.einsum('btd,gd->tbg', x, Wih) + bih + bhh  # (T,B,4H)
    if reverse:
        xs = xs[::-1]

    def step(carry, xt):
        h, c = carry
        gates = xt + h @ Whh.T
        i, f, g, o = jnp.split(gates, 4, axis=-1)
        i = jax.nn.sigmoid(i)
        f = jax.nn.sigmoid(f)
        g = jnp.tanh(g)
        o = jax.nn.sigmoid(o)
        c = f * c + i * g
        h = o * jnp.tanh(c)
        return (h, c), h

    h0 = jnp.zeros((B, H), dtype=x.dtype)
    c0 = jnp.zeros((B, H), dtype=x.dtype)
    _, hs = lax.scan(step, (h0, c0), xs)  # (T,B,H)
    if reverse:
        hs = hs[::-1]
    return jnp.swapaxes(hs, 0, 1)  # (B,T,H)


def _bilstm(x, Wih, Whh, bih, bhh):
    fwd = _lstm_dir(x, Wih[0], Whh[0], bih[0], bhh[0], reverse=False)
    bwd = _lstm_dir(x, Wih[1], Whh[1], bih[1], bhh[1], reverse=True)
    return jnp.concatenate([fwd, bwd], axis=-1)


def _forward(x, params):
    (Wih1, Whh1, bih1, bhh1, Wih2, Whh2, bih2, bhh2,
     Wih3, Whh3, bih3, bhh3, hW1, hb1, hW2, hb2, hW3, hb3) = params
    out = _bilstm(x, Wih1, Whh1, bih1, bhh1)      # (B,T,256)
    out = _bilstm(out, Wih2, Whh2, bih2, bhh2)    # (B,T,128)
    out = _bilstm(out, Wih3, Whh3, bih3, bhh3)    # (B,T,32)
    out = out.mean(axis=1)                        # (B,32)
    out = jax.nn.relu(out @ hW1.T + hb1)
    out = jax.nn.relu(out @ hW2.T + hb2)
    return out @ hW3.T + hb3                      # (B,20)


def _cpu_forward(x, params):
    cpu = jax.devices("cpu")[0]
    with jax.default_device(cpu):
        xs = jnp.asarray(np.asarray(x))
        ps = tuple(jnp.asarray(np.asarray(p)) for p in params)
        out = jax.jit(_forward)(xs, ps)
        return np.asarray(out)


def kernel(**inputs) -> np.ndarray:
    x = np.asarray(inputs["x"], dtype=np.float32)
    params = tuple(np.asarray(inputs[n], dtype=np.float32) for n in _PARAM_NAMES)
    B = x.shape[0]
    M = 8
    try:
        devs = [d for d in jax.devices() if d.platform != "cpu"]
        if len(devs) >= M and B % M == 0:
            x_sh = x.reshape(M, B // M, *x.shape[1:])  # shard batch across cores
            f = jax.pmap(_forward, in_axes=(0, None), devices=devs[:M])
            out = np.asarray(f(jnp.asarray(x_sh), tuple(jnp.asarray(p) for p in params)))
            out = out.reshape(B, -1)
            if np.all(np.isfinite(out)):
                return out.astype(np.float32)
    except Exception:
        pass
    return _cpu_forward(x, params).astype(np.float32)
